# revision 1
# baseline (speedup 1.0000x reference)
"""Trainium2 Bass kernel for DQLinearLoRA (NF4-style blockwise dequant + LoRA linear).

Computes out = x @ dequant(weight).T + (x @ lora_A.T) @ lora_B.T on 8 NeuronCores.

Sharding: tensor-parallel over out_features (each core owns 512 of 4096 rows of
weight / lora_B / max_val blocks); x is replicated. Each core:
  1. dequantizes its weight slice on-chip (DVE staircase: 15 threshold compares
     against codebook midpoints, scaled back by the per-block absmax),
  2. merges the LoRA update (lora_B @ lora_A, computed by TensorE) into the
     dequantized weight slab held in SBUF,
  3. streams x.T tiles from HBM through TensorE against the resident slab,
     producing its out.T slice.
Host side only reshapes/transposes/concatenates (layout prep for sharding).
"""

import sys
from contextlib import ExitStack

import numpy as np

sys.path.insert(0, "/opt/trn_rl_repo")

import concourse.bacc as bacc
import concourse.mybir as mybir
from concourse import tile
from concourse.bass_utils import run_bass_kernel_spmd

P = 128  # partitions
BLOCK = 64  # quantization block size

# Problem dims (hardcoded per contract)
T_FULL = 8192
IN_F = 4096
OUT_F = 4096
RANK = 64
N_CORES = 8

# mode: "f32" (exact, 4 cyc/row matmul) | "bf16" | "f32r"
MODE = "f32r"
N_ACT = 0  # PE now sums the masks; direct DVE compares beat the ACT Sign detour
N_GP = 0  # GpSimd accumulate disabled: Pool TS measured 8.4us/op on HW

_CACHE = {}


def _np_dt(dt):
    return np.dtype(mybir.dt.np(dt))


def build_program(T, IF, OPC, R, n_cores, mids, deltas, c0, mode, t_tile=512):
    """Build the per-core SPMD program. mids/deltas/c0: python floats baked in."""
    f32 = mybir.dt.float32
    bf16 = mybir.dt.bfloat16
    if mode == "bf16":
        dt_x = bf16  # x.T storage/matmul dtype
        dt_q = bf16  # dequant accumulator / qweight slab dtype
        dt_sc = bf16  # maxB scale tile dtype
    else:
        dt_x = f32
        dt_q = f32
        dt_sc = f32
    # dequant engine split (f32r/bf16: spread staircase over DVE+ACT+GPSIMD so
    # its span shrinks below the PE matmul span; f32 mode is PE-bound anyway)
    if mode == "f32":
        n_act = 0
        n_gp = 0
        dt_acc = f32
    else:
        n_act = N_ACT  # levels evaluated as ACT Sign(u - m_j)
        n_gp = N_GP  # of those, how many are accumulated on GpSimd (TS+TT pairs)
        dt_acc = bf16
    if mode == "f32r":
        # float32r: storage is fp32-width but the verifier requires producer
        # ops to declare the rounded type, so the x path and qw slab are
        # declared float32r end-to-end (numpy side still float32).
        dt_x = mybir.dt.float32r
        dt_q = mybir.dt.float32r

    KT = IF // P  # k tiles
    OS = OPC // P  # out-feature 128-slices per core
    NTT = T // t_tile  # token tiles
    NLVL = len(mids)  # 15

    nc = bacc.Bacc(
        "TRN2",
        target_bir_lowering=False,
        debug=False,
        num_devices=n_cores,
    )
    op = mybir.AluOpType

    if n_act:
        # ACT activation biases must exist as const APs
        for j in range(NLVL - n_act, NLVL):
            v = -float(mids[j])
            key = (f32, v)
            if key not in nc.const_aps.aps:
                t = nc.alloc_sbuf_tensor(f"const-f32-m{j}", [P, 1], f32)
                nc.gpsimd.memset(t.ap(), v)
                nc.const_aps.aps[key] = t.ap()
        nc.all_engine_barrier()

    ident = nc.dram_tensor("ident", [P, P], bf16, kind="ExternalInput").ap()
    xT = nc.dram_tensor("xT", [IF, T], dt_x, kind="ExternalInput").ap()
    wT = nc.dram_tensor("wT", [IF, OPC], f32, kind="ExternalInput").ap()
    maxB = nc.dram_tensor("maxB", [IF, OPC], dt_sc, kind="ExternalInput").ap()
    rB = nc.dram_tensor("rB", [IF, OPC], f32, kind="ExternalInput").ap()
    A = nc.dram_tensor("A", [R, IF], f32, kind="ExternalInput").ap()
    BT = nc.dram_tensor("BT", [R, OPC], f32, kind="ExternalInput").ap()
    outT = nc.dram_tensor("outT", [OPC, T], f32, kind="ExternalOutput").ap()

    with tile.TileContext(nc) as tc, ExitStack() as ctx:
        const = ctx.enter_context(tc.tile_pool(name="const", bufs=1))
        A_sb = const.tile([R, IF], f32)
        nc.sync.dma_start(A_sb[:], A[:])
        BT_sb = const.tile([R, OPC], f32)
        nc.sync.dma_start(BT_sb[:], BT[:])
        id_sb = const.tile([P, P], bf16, name="id_sb")
        nc.sync.dma_start(id_sb[:], ident[:])

        if n_gp:
            one_tile = const.tile([P, 1], f32, name="one_c")
            nc.vector.memset(one_tile[:], 1.0)

        qw_pool = ctx.enter_context(tc.tile_pool(name="qw", bufs=KT))
        wrk = ctx.enter_context(tc.tile_pool(name="wrk", bufs=4))
        accp = ctx.enter_context(tc.tile_pool(name="accp", bufs=2))
        psum = ctx.enter_context(tc.tile_pool(name="psum", bufs=6, space="PSUM"))
        dqps = ctx.enter_context(tc.tile_pool(name="dqps", bufs=2, space="PSUM"))
        bap = ctx.enter_context(tc.tile_pool(name="bap", bufs=12))

        # ---- Phase L: all LoRA slab tiles first — dense PE work at t=0,
        # evicted to SBUF f32 so no PSUM bank is held during dequant.
        # (lora_B @ lora_A).T[ksl, :] = A[:, ksl].T @ BT
        ba_tiles = []
        for kt in range(KT):
            ksl = slice(kt * P, (kt + 1) * P)
            ba_ps = psum.tile([P, OPC], f32, tag="ps", name=f"baps{kt}")
            nc.tensor.matmul(ba_ps[:], A_sb[:, ksl], BT_sb[:], start=True, stop=True)
            ba_sb = bap.tile([P, OPC], f32, tag="ba", name=f"ba{kt}")
            nc.scalar.copy(ba_sb[:], ba_ps[:])
            ba_tiles.append(ba_sb)

        # ---- Phase D: dequant weight slice, one [128, OPC] k-tile at a time
        qw_tiles = []
        for kt in range(KT):
            ksl = slice(kt * P, (kt + 1) * P)
            w_sb = wrk.tile([P, OPC], f32, tag="w")
            nc.sync.dma_start(w_sb[:], wT[ksl, :])
            rb_sb = wrk.tile([P, OPC], f32, tag="rb")
            nc.sync.dma_start(rb_sb[:], rB[ksl, :])
            mx_sb = wrk.tile([P, OPC], dt_sc, tag="mx")
            nc.sync.dma_start(mx_sb[:], maxB[ksl, :])

            # u = w / max  (via reciprocal precomputed on host)
            u_sb = wrk.tile([P, OPC], f32, tag="u")
            nc.vector.tensor_tensor(u_sb[:], w_sb[:], rb_sb[:], op=op.mult)

            # staircase: sum_j (u > mids[j]) * deltas[j].
            # DVE/ACT produce scaled mask tiles t_j; their SUM runs on TensorE
            # as identity matmuls accumulating in PSUM (f32, exact), freeing
            # DVE from the 14-add chain. f32 mode keeps the all-DVE f32 chain.
            dve_lv = list(range(NLVL - n_act))
            act_lv = list(range(NLVL - n_act, NLVL))
            c0_eff = float(c0)

            def mk_tj(j):
                # scaled mask tile (u > m_j) * d_j in dt_acc
                if j in act_lv:
                    sg = wrk.tile([P, OPC], dt_acc, tag="sg", name=f"sg{kt}_{j}")
                    nc.scalar.activation(
                        sg[:],
                        u_sb[:],
                        mybir.ActivationFunctionType.Sign,
                        bias=-float(mids[j]),
                    )
                    tj = wrk.tile([P, OPC], dt_acc, tag="tj", name=f"tjs{kt}_{j}")
                    nc.vector.tensor_scalar(
                        tj[:], sg[:], 0.0, float(deltas[j]), op0=op.is_gt, op1=op.mult
                    )
                else:
                    tj = wrk.tile([P, OPC], dt_acc, tag="tj", name=f"tjc{kt}_{j}")
                    nc.vector.tensor_scalar(
                        tj[:], u_sb[:], float(mids[j]), float(deltas[j]),
                        op0=op.is_gt, op1=op.mult,
                    )
                return tj

            if mode == "f32":
                acc = accp.tile([P, OPC], dt_acc, tag="acc")
                nc.vector.tensor_scalar(
                    acc[:], u_sb[:], float(mids[0]), float(deltas[0]),
                    op0=op.is_gt, op1=op.mult,
                )
                for j in range(1, NLVL):
                    tj = mk_tj(j)
                    nc.vector.tensor_tensor(acc[:], acc[:], tj[:], op=op.add)
                qsc = wrk.tile([P, OPC], dt_sc, tag="qsc")
                nc.vector.scalar_tensor_tensor(
                    qsc[:], acc[:], c0_eff, mx_sb[:], op0=op.add, op1=op.mult
                )
            else:
                dq_ps = dqps.tile([P, OPC], f32, tag="dq", name=f"dq{kt}")
                for i in range(NLVL):
                    tj = mk_tj(i)
                    nc.tensor.matmul(
                        dq_ps[:], id_sb[:], tj[:], start=(i == 0), stop=(i == NLVL - 1)
                    )
                qsc = wrk.tile([P, OPC], dt_sc, tag="qsc")
                nc.vector.scalar_tensor_tensor(
                    qsc[:], dq_ps[:], c0_eff, mx_sb[:], op0=op.add, op1=op.mult
                )
            # qw = qsc + (lora_B@lora_A).T tile
            qw_sb = qw_pool.tile([P, OPC], dt_q, tag="qwt")
            nc.vector.tensor_tensor(qw_sb[:], qsc[:], ba_tiles[kt][:], op=op.add)
            qw_tiles.append(qw_sb)

        # ---- Phase M: backbone matmul, out.T[o, t] tiles, two token-tiles per
        # batch (8 PSUM chains) so PE can consume dequant output incrementally.
        xp = ctx.enter_context(tc.tile_pool(name="xp", bufs=8))
        ob = ctx.enter_context(tc.tile_pool(name="ob", bufs=4))
        TB = 1
        for tb in range(0, NTT, TB):
            tts = list(range(tb, min(tb + TB, NTT)))
            ps = {
                (tt, o): psum.tile([P, t_tile], f32, tag="ps", name=f"ps{tt}_{o}")
                for tt in tts
                for o in range(OS)
            }
            for kt in range(KT):
                xs = {}
                for tt in tts:
                    x_sb = xp.tile([P, t_tile], dt_x, tag="x", name=f"x{tt}_{kt}")
                    tsl = slice(tt * t_tile, (tt + 1) * t_tile)
                    nc.sync.dma_start(x_sb[:], xT[kt * P : (kt + 1) * P, tsl])
                    xs[tt] = x_sb
                for tt in tts:
                    for o in range(OS):
                        nc.tensor.matmul(
                            ps[(tt, o)][:],
                            qw_tiles[kt][:, o * P : (o + 1) * P],
                            xs[tt][:],
                            start=(kt == 0),
                            stop=(kt == KT - 1),
                        )
            for tt in tts:
                tsl = slice(tt * t_tile, (tt + 1) * t_tile)
                for o in range(OS):
                    o_sb = ob.tile([P, t_tile], f32, tag="osb", name=f"ob{tt}_{o}")
                    nc.scalar.copy(o_sb[:], ps[(tt, o)][:])
                    nc.sync.dma_start(outT[o * P : (o + 1) * P, tsl], o_sb[:])

    nc.compile()
    return nc


def _lut_consts(lookup_table):
    lut = np.asarray(lookup_table, np.float64)
    mids = ((lut[:-1] + lut[1:]) / 2).astype(np.float32)
    deltas = (lut[1:] - lut[:-1]).astype(np.float32)
    c0 = np.float32(lut[0])
    return mids, deltas, c0


def prep_inputs(x, weight, lora_A, lora_B, max_val, mode, n_cores=N_CORES):
    """Host-side sharding/layout prep. Returns in_maps (one dict per core)."""
    f32 = np.float32
    T, IF = x.shape
    OF = weight.shape[0]
    OPC = OF // n_cores
    dt_x = _np_dt(mybir.dt.bfloat16) if mode == "bf16" else f32
    dt_sc = dt_x if mode == "bf16" else f32

    xT = np.ascontiguousarray(np.asarray(x, f32).T).astype(dt_x)
    A = np.ascontiguousarray(np.asarray(lora_A, f32))
    maxR = np.asarray(max_val, f32).reshape(OF, IF // BLOCK)  # [o, block]
    w = np.asarray(weight, f32)
    B = np.asarray(lora_B, f32)

    in_maps = []
    for c in range(n_cores):
        osl = slice(c * OPC, (c + 1) * OPC)
        wT_c = np.ascontiguousarray(w[osl].T)  # [IF, OPC]
        mx_c = np.repeat(maxR[osl].T, BLOCK, axis=0)  # [IF, OPC]
        rb_c = (f32(1.0) / mx_c).astype(f32)
        in_maps.append(
            {
                "ident": np.eye(P, dtype=_np_dt(mybir.dt.bfloat16)),
                "xT": xT,
                "wT": wT_c,
                "maxB": mx_c.astype(dt_sc),
                "rB": rb_c,
                "A": A,
                "BT": np.ascontiguousarray(B[osl].T),  # [R, OPC]
            }
        )
    return in_maps


def _get_program(mids, deltas, c0, mode):
    key = (mode, tuple(np.asarray(mids).tolist()), tuple(np.asarray(deltas).tolist()), float(c0))
    if key not in _CACHE:
        _CACHE[key] = build_program(
            T_FULL, IN_F, OUT_F // N_CORES, RANK, N_CORES, mids, deltas, c0, mode
        )
    return _CACHE[key]


def kernel(x, weight, lora_A, lora_B, max_val, lookup_table):
    mids, deltas, c0 = _lut_consts(lookup_table)
    nc = _get_program(mids, deltas, c0, MODE)
    in_maps = prep_inputs(x, weight, lora_A, lora_B, max_val, MODE)
    res = run_bass_kernel_spmd(nc, in_maps, core_ids=list(range(N_CORES))).results
    outT = np.concatenate([res[c]["outT"] for c in range(N_CORES)], axis=0)  # [OF, T]
    return np.ascontiguousarray(outT.T).astype(np.float32)



# revision 9
# speedup vs baseline: 1.0118x; 1.0118x over previous
"""Trainium2 Bass kernel for DQLinearLoRA (NF4-style blockwise dequant + LoRA linear).

Computes out = x @ dequant(weight).T + (x @ lora_A.T) @ lora_B.T on 8 NeuronCores.

Sharding: tensor-parallel over out_features (each core owns 512 of 4096 rows of
weight / lora_B / max_val blocks); x is replicated. Per core:

  1. Dequant staircase on u = w/max (bf16): 11 threshold levels as DVE
     tensor_scalar compares (4x-rate bf16 masks, delta-scaled), 4 levels as ACT
     Sign ops; all 15 summed by TensorE identity-matmuls into a PSUM bank
     (delta/2-scaled identities apply the sign-level weights for free).
  2. LoRA slab (lora_B @ lora_A).T produced by TensorE in bf16, merged with the
     dequantized tile into a resident bf16 weight slab.
  3. Backbone matmul streams bf16 x tiles against the slab (bf16 -> FWL weight
     loads, 1 col/cycle) into 5 concurrent PSUM chains; dequant of ktile k
     overlaps pass-0 matmuls of ktile k-1 so the PE never idles.

Host side only reshapes/transposes/casts (layout prep for sharding).
"""

import sys
from contextlib import ExitStack

import numpy as np

sys.path.insert(0, "/opt/trn_rl_repo")

import concourse.bacc as bacc
import concourse.mybir as mybir
from concourse import tile
from concourse.bass_utils import run_bass_kernel_spmd

P = 128  # partitions
BLOCK = 64  # quantization block size

# Problem dims (hardcoded per contract)
T_FULL = 8192
IN_F = 4096
OUT_F = 4096
RANK = 64
N_CORES = 8

MODE = "v2"
N_DVE = 11  # staircase levels on DVE (rest go to ACT as Sign)
CH = 5  # concurrent PSUM output chains (5 + 2 dequant + 1 lora = 8 banks)

_CACHE = {}


def _np_dt(dt):
    return np.dtype(mybir.dt.np(dt))


def build_program(T, IF, OPC, R, n_cores, mids, deltas, c0, mode, t_tile=512):
    """Build the per-core SPMD program. mids/deltas/c0: python floats baked in."""
    f32 = mybir.dt.float32
    bf16 = mybir.dt.bfloat16
    KT = IF // P  # k tiles (32)
    OS = OPC // P  # out-feature 128-slices per core (4)
    NTT = T // t_tile  # token tiles (16)
    NLVL = len(mids)  # 15
    dve_lv = list(range(N_DVE))
    act_lv = list(range(N_DVE, NLVL))
    # Sign levels contribute delta_j/2*(sign+1); the +1 halves fold into C0.
    C0 = float(c0) + sum(float(deltas[j]) / 2.0 for j in act_lv)
    NID = 1 + len(act_lv)  # identity stack: plain I + delta_j/2-scaled per sign level

    nc = bacc.Bacc(
        "TRN2",
        target_bir_lowering=False,
        debug=False,
        num_devices=n_cores,
    )
    op = mybir.AluOpType

    # ACT activation biases must exist as const APs before the tile context.
    for j in act_lv:
        v = -float(mids[j])
        key = (f32, v)
        if key not in nc.const_aps.aps:
            t_ = nc.alloc_sbuf_tensor(f"const-f32-m{j}", [P, 1], f32)
            nc.gpsimd.memset(t_.ap(), v)
            nc.const_aps.aps[key] = t_.ap()
    nc.all_engine_barrier()

    ident = nc.dram_tensor("ident", [P, P], bf16, kind="ExternalInput").ap()
    xT = nc.dram_tensor("xT", [IF, T], bf16, kind="ExternalInput").ap()
    wT = nc.dram_tensor("wT", [IF, OPC], f32, kind="ExternalInput").ap()
    maxB = nc.dram_tensor("maxB", [IF, OPC], f32, kind="ExternalInput").ap()
    rB = nc.dram_tensor("rB", [IF, OPC], f32, kind="ExternalInput").ap()
    A = nc.dram_tensor("A", [R, IF], bf16, kind="ExternalInput").ap()
    BT = nc.dram_tensor("BT", [R, OPC], bf16, kind="ExternalInput").ap()
    outT = nc.dram_tensor("outT", [OPC, T], f32, kind="ExternalOutput").ap()

    chunks = [(tt, o) for tt in range(NTT) for o in range(OS)]  # tt-major, 64
    p0 = chunks[:CH]
    groups = [chunks[i : i + CH] for i in range(CH, len(chunks), CH)]
    p0_tts = sorted({tt for tt, _ in p0})

    with tile.TileContext(nc) as tc, ExitStack() as ctx:
        const = ctx.enter_context(tc.tile_pool(name="const", bufs=1))
        A_sb = const.tile([R, IF], bf16, name="A_sb")
        nc.sync.dma_start(A_sb[:], A[:])
        BT_sb = const.tile([R, OPC], bf16, name="BT_sb")
        nc.sync.dma_start(BT_sb[:], BT[:])
        id_sb = const.tile([P, NID * P], bf16, name="id_sb")
        nc.sync.dma_start(id_sb[:, 0:P], ident[:])
        # delta_j/2-scaled identities for the Sign levels, built on-chip
        for bi, j in enumerate(act_lv):
            nc.vector.tensor_scalar(
                id_sb[:, (1 + bi) * P : (2 + bi) * P],
                id_sb[:, 0:P],
                float(deltas[j]) / 2.0,
                None,
                op0=op.mult,
            )

        wrk = ctx.enter_context(tc.tile_pool(name="wrk", bufs=3))
        ub = ctx.enter_context(tc.tile_pool(name="ub", bufs=3))
        mk = ctx.enter_context(tc.tile_pool(name="mk", bufs=2 * NLVL + 2))
        qs = ctx.enter_context(tc.tile_pool(name="qs", bufs=2))
        qwp = ctx.enter_context(tc.tile_pool(name="qwp", bufs=KT))
        bab = ctx.enter_context(tc.tile_pool(name="bab", bufs=3))
        xp = ctx.enter_context(tc.tile_pool(name="xp", bufs=12))
        obp = ctx.enter_context(tc.tile_pool(name="obp", bufs=6))
        cps = ctx.enter_context(tc.tile_pool(name="cps", bufs=CH, space="PSUM"))
        dps = ctx.enter_context(tc.tile_pool(name="dps", bufs=2, space="PSUM"))
        bps = ctx.enter_context(tc.tile_pool(name="bps", bufs=1, space="PSUM"))

        # ---- weight-staging and x-tile DMA helpers (2-iteration prefetch)
        stage = {}

        def emit_wstage(kt):
            w_sb = wrk.tile([P, OPC], f32, tag="w", name=f"w{kt}")
            nc.sync.dma_start(w_sb[:], wT[kt * P : (kt + 1) * P, :])
            rb_sb = wrk.tile([P, OPC], f32, tag="rb", name=f"rb{kt}")
            nc.sync.dma_start(rb_sb[:], rB[kt * P : (kt + 1) * P, :])
            mx_sb = wrk.tile([P, OPC], f32, tag="mx", name=f"mx{kt}")
            nc.sync.dma_start(mx_sb[:], maxB[kt * P : (kt + 1) * P, :])
            stage[kt] = (w_sb, rb_sb, mx_sb)

        def emit_x(store, kt, tts, label):
            for tt in tts:
                if (kt, tt) not in store:
                    xt = xp.tile([P, t_tile], bf16, tag="x", name=f"x{label}_{kt}_{tt}")
                    nc.sync.dma_start(
                        xt[:], xT[kt * P : (kt + 1) * P, tt * t_tile : (tt + 1) * t_tile]
                    )
                    store[(kt, tt)] = xt

        emit_wstage(0)
        emit_wstage(1)
        x0 = {}
        emit_x(x0, 0, p0_tts, "p0")

        ps0 = {
            c: cps.tile([P, t_tile], f32, tag="ps", name=f"ps0_{c[0]}_{c[1]}")
            for c in p0
        }
        qw_tiles = [None] * KT
        dq_st = [None] * KT
        ba_tiles = [None] * KT

        # ---- Phase 0: 2-deep pipeline — dequant ktile kt, finalize slab for
        # kt-1, pass-0 matmuls for kt-2. PE order per iter: lora(kt),
        # p0mm(kt-2), dve-level idmuls(kt), act-level idmuls(kt) — timed so PE
        # never waits on DVE mask / ACT sign production.
        for it in range(KT + 2):
            kt = it
            if kt < KT:
                if kt + 2 < KT:
                    emit_wstage(kt + 2)
                if kt + 1 < KT:
                    emit_x(x0, kt + 1, p0_tts, "p0")
                w_sb, rb_sb, mx_sb = stage.pop(kt)
                ksl = slice(kt * P, (kt + 1) * P)
                # LoRA tile: (lora_B @ lora_A).T[ksl, :] = A[:, ksl].T @ BT
                ba_ps = bps.tile([P, OPC], f32, tag="ba", name=f"baps{kt}")
                nc.tensor.matmul(ba_ps[:], A_sb[:, ksl], BT_sb[:], start=True, stop=True)
                # u = w / max (reciprocal precomputed on host), rounded to bf16
                u_sb = ub.tile([P, OPC], bf16, tag="u", name=f"u{kt}")
                nc.vector.tensor_tensor(u_sb[:], w_sb[:], rb_sb[:], op=op.mult)
                # staircase masks: DVE levels emit delta_j*(u > m_j) in bf16 (4x)
                lvl = []
                for j in dve_lv:
                    m = mk.tile([P, OPC], bf16, tag="mk", name=f"m{kt}_{j}")
                    nc.vector.tensor_scalar(
                        m[:], u_sb[:], float(mids[j]), float(deltas[j]),
                        op0=op.is_gt, op1=op.mult,
                    )
                    lvl.append((m, 0))
                # ACT levels: sign(u - m_j); delta_j/2 applied by scaled identity
                for bi, j in enumerate(act_lv):
                    s = mk.tile([P, OPC], bf16, tag="mk", name=f"s{kt}_{j}")
                    nc.scalar.activation(
                        s[:], u_sb[:], mybir.ActivationFunctionType.Sign,
                        bias=-float(mids[j]),
                    )
                    lvl.append((s, 1 + bi))
                # LoRA eviction on ACT after the signs (PSUM -> bf16 SBUF)
                ba_sb = bab.tile([P, OPC], bf16, tag="ba", name=f"ba{kt}")
                nc.scalar.copy(ba_sb[:], ba_ps[:])
                ba_tiles[kt] = ba_sb
            # pass-0 matmuls for kt-2 (PE-early: fills the mask-latency window)
            if it >= 2:
                pk2 = it - 2
                qwt2 = qw_tiles[pk2]
                for tt, o in p0:
                    nc.tensor.matmul(
                        ps0[(tt, o)][:],
                        qwt2[:, o * P : (o + 1) * P],
                        x0[(pk2, tt)][:],
                        start=(pk2 == 0),
                        stop=(pk2 == KT - 1),
                    )
                for tt in p0_tts:
                    x0.pop((pk2, tt), None)
            if kt < KT:
                # PE sums all level tiles into the dequant PSUM bank
                dq = dps.tile([P, OPC], f32, tag="dq", name=f"dq{kt}")
                for i, (m, blk) in enumerate(lvl):
                    nc.tensor.matmul(
                        dq[:], id_sb[:, blk * P : (blk + 1) * P], m[:],
                        start=(i == 0), stop=(i == len(lvl) - 1),
                    )
                dq_st[kt] = (dq, mx_sb)
            if 1 <= it <= KT:
                pk = it - 1
                dq, mx_sb = dq_st[pk]
                dq_st[pk] = None
                # qsc = (sum + C0) * max, then merge LoRA -> resident bf16 slab
                qsc = qs.tile([P, OPC], bf16, tag="qsc", name=f"qsc{pk}")
                nc.vector.scalar_tensor_tensor(
                    qsc[:], dq[:], C0, mx_sb[:], op0=op.add, op1=op.mult
                )
                qwt = qwp.tile([P, OPC], bf16, tag="qwt", name=f"qw{pk}")
                nc.vector.tensor_tensor(qwt[:], qsc[:], ba_tiles[pk][:], op=op.add)
                ba_tiles[pk] = None
                qw_tiles[pk] = qwt

        for tt, o in p0:
            o_sb = obp.tile([P, t_tile], f32, tag="o", name=f"ob0_{tt}_{o}")
            nc.scalar.copy(o_sb[:], ps0[(tt, o)][:])
            nc.sync.dma_start(
                outT[o * P : (o + 1) * P, tt * t_tile : (tt + 1) * t_tile], o_sb[:]
            )

        # ---- Remaining passes: CH chunks each, slab resident, x streamed
        steps = [(gi, kt) for gi in range(len(groups)) for kt in range(KT)]
        stores = [dict() for _ in groups]
        g_tts = [sorted({tt for tt, _ in g}) for g in groups]

        def prefetch(si):
            if si < len(steps):
                gi2, kt2 = steps[si]
                emit_x(stores[gi2], kt2, g_tts[gi2], f"g{gi2}")

        prefetch(0)
        prefetch(1)
        cur_ps = {}
        for si, (gi, kt) in enumerate(steps):
            if kt == 0:
                cur_ps = {
                    c: cps.tile([P, t_tile], f32, tag="ps", name=f"ps{gi}_{c[0]}_{c[1]}")
                    for c in groups[gi]
                }
            prefetch(si + 2)
            st = stores[gi]
            for tt, o in groups[gi]:
                nc.tensor.matmul(
                    cur_ps[(tt, o)][:],
                    qw_tiles[kt][:, o * P : (o + 1) * P],
                    st[(kt, tt)][:],
                    start=(kt == 0),
                    stop=(kt == KT - 1),
                )
            for tt in g_tts[gi]:
                st.pop((kt, tt), None)
            if kt == KT - 1:
                for tt, o in groups[gi]:
                    o_sb = obp.tile([P, t_tile], f32, tag="o", name=f"obg{gi}_{tt}_{o}")
                    nc.scalar.copy(o_sb[:], cur_ps[(tt, o)][:])
                    nc.sync.dma_start(
                        outT[o * P : (o + 1) * P, tt * t_tile : (tt + 1) * t_tile],
                        o_sb[:],
                    )

    nc.compile()
    return nc


def _lut_consts(lookup_table):
    lut = np.asarray(lookup_table, np.float64)
    mids = ((lut[:-1] + lut[1:]) / 2).astype(np.float32)
    deltas = (lut[1:] - lut[:-1]).astype(np.float32)
    c0 = np.float32(lut[0])
    return mids, deltas, c0


def prep_inputs(x, weight, lora_A, lora_B, max_val, mode, n_cores=N_CORES):
    """Host-side sharding/layout prep. Returns in_maps (one dict per core)."""
    f32 = np.float32
    bf16 = _np_dt(mybir.dt.bfloat16)
    T, IF = x.shape
    OF = weight.shape[0]
    OPC = OF // n_cores

    xT = np.ascontiguousarray(np.asarray(x, f32).T).astype(bf16)
    A = np.ascontiguousarray(np.asarray(lora_A, f32)).astype(bf16)
    maxR = np.asarray(max_val, f32).reshape(OF, IF // BLOCK)  # [o, block]
    w = np.asarray(weight, f32)
    B = np.asarray(lora_B, f32)

    in_maps = []
    for c in range(n_cores):
        osl = slice(c * OPC, (c + 1) * OPC)
        wT_c = np.ascontiguousarray(w[osl].T)  # [IF, OPC]
        mx_c = np.repeat(maxR[osl].T, BLOCK, axis=0)  # [IF, OPC]
        rb_c = (f32(1.0) / mx_c).astype(f32)
        in_maps.append(
            {
                "ident": np.eye(P, dtype=bf16),
                "xT": xT,
                "wT": wT_c,
                "maxB": mx_c.astype(f32),
                "rB": rb_c,
                "A": A,
                "BT": np.ascontiguousarray(B[osl].T).astype(bf16),  # [R, OPC]
            }
        )
    return in_maps


def _get_program(mids, deltas, c0, mode):
    key = (
        mode,
        tuple(np.asarray(mids).tolist()),
        tuple(np.asarray(deltas).tolist()),
        float(c0),
    )
    if key not in _CACHE:
        _CACHE[key] = build_program(
            T_FULL, IN_F, OUT_F // N_CORES, RANK, N_CORES, mids, deltas, c0, mode
        )
    return _CACHE[key]


def kernel(x, weight, lora_A, lora_B, max_val, lookup_table):
    mids, deltas, c0 = _lut_consts(lookup_table)
    nc = _get_program(mids, deltas, c0, MODE)
    in_maps = prep_inputs(x, weight, lora_A, lora_B, max_val, MODE)
    res = run_bass_kernel_spmd(nc, in_maps, core_ids=list(range(N_CORES))).results
    outT = np.concatenate([res[c]["outT"] for c in range(N_CORES)], axis=0)  # [OF, T]
    return np.ascontiguousarray(outT.T).astype(np.float32)


# revision 10
# speedup vs baseline: 1.2728x; 1.2580x over previous
"""Trainium2 Bass kernel for DQLinearLoRA (NF4-style blockwise dequant + LoRA linear).

Computes out = x @ dequant(weight).T + (x @ lora_A.T) @ lora_B.T on 8 NeuronCores.

Sharding: tensor-parallel over out_features (each core owns 512 of 4096 rows of
weight / lora_B / max_val blocks); x is replicated. Per core:

  1. Dequant staircase on u = w/max (bf16): 11 threshold levels as DVE
     tensor_scalar compares (4x-rate bf16 masks, delta-scaled), 4 levels as ACT
     Sign ops; all 15 summed by TensorE identity-matmuls into a PSUM bank
     (delta/2-scaled identities apply the sign-level weights for free).
  2. LoRA slab (lora_B @ lora_A).T produced by TensorE in bf16, merged with the
     dequantized tile into a resident bf16 weight slab.
  3. Backbone matmul streams bf16 x tiles against the slab (bf16 -> FWL weight
     loads): 4 PSUM chains overlap the dequant pipeline (pass 0), then the
     dequant PSUM banks are released and the remaining passes run 8 chains with
     one wide x DMA per ktile. PSUM evictions run on the otherwise-idle DVE and
     out-DMAs are queued behind the next pass's x prefetches (no head-of-line
     blocking on the sync DMA queue).

Host side only reshapes/transposes/casts (layout prep for sharding).
"""

import sys
from contextlib import ExitStack

import numpy as np

sys.path.insert(0, "/opt/trn_rl_repo")

import concourse.bacc as bacc
import concourse.mybir as mybir
from concourse import tile
from concourse.bass_utils import run_bass_kernel_spmd

P = 128  # partitions
BLOCK = 64  # quantization block size

# Problem dims (hardcoded per contract)
T_FULL = 8192
IN_F = 4096
OUT_F = 4096
RANK = 64
N_CORES = 8

MODE = "v3"
N_DVE = 11  # staircase levels on DVE (rest go to ACT as Sign)
CH0 = 4  # pass-0 PSUM chains (+3 dequant +1 lora = 8 banks)
CH = 8  # post-phase PSUM chains (dequant banks released)

_CACHE = {}


def _np_dt(dt):
    return np.dtype(mybir.dt.np(dt))


def build_program(T, IF, OPC, R, n_cores, mids, deltas, c0, mode, t_tile=512):
    """Build the per-core SPMD program. mids/deltas/c0: python floats baked in."""
    f32 = mybir.dt.float32
    bf16 = mybir.dt.bfloat16
    KT = IF // P  # k tiles (32)
    OS = OPC // P  # out-feature 128-slices per core (4)
    NTT = T // t_tile  # token tiles (16)
    NLVL = len(mids)  # 15
    dve_lv = list(range(N_DVE))
    act_lv = list(range(N_DVE, NLVL))
    # Sign levels contribute delta_j/2*(sign+1); the +1 halves fold into C0.
    C0 = float(c0) + sum(float(deltas[j]) / 2.0 for j in act_lv)
    NID = 1 + len(act_lv)  # identity stack: plain I + delta_j/2-scaled per sign level

    nc = bacc.Bacc(
        "TRN2",
        target_bir_lowering=False,
        debug=False,
        num_devices=n_cores,
    )
    op = mybir.AluOpType

    # ACT activation biases must exist as const APs before the tile context.
    for j in act_lv:
        v = -float(mids[j])
        key = (f32, v)
        if key not in nc.const_aps.aps:
            t_ = nc.alloc_sbuf_tensor(f"const-f32-m{j}", [P, 1], f32)
            nc.gpsimd.memset(t_.ap(), v)
            nc.const_aps.aps[key] = t_.ap()
    nc.all_engine_barrier()

    ident = nc.dram_tensor("ident", [P, P], bf16, kind="ExternalInput").ap()
    xT = nc.dram_tensor("xT", [IF, T], bf16, kind="ExternalInput").ap()
    # packed per-ktile dequant inputs: [w | 1/max | max], each OPC wide
    wrm = nc.dram_tensor("wrm", [IF, 3 * OPC], f32, kind="ExternalInput").ap()
    A = nc.dram_tensor("A", [R, IF], bf16, kind="ExternalInput").ap()
    BT = nc.dram_tensor("BT", [R, OPC], bf16, kind="ExternalInput").ap()
    outT = nc.dram_tensor("outT", [OPC, T], f32, kind="ExternalOutput").ap()

    chunks = [(tt, o) for tt in range(NTT) for o in range(OS)]  # tt-major, 64
    p0 = chunks[:CH0]  # (tt0, o0..3)
    rest = chunks[CH0:]
    groups = [rest[i : i + CH] for i in range(0, len(rest), CH)]  # 2-tt aligned

    with tile.TileContext(nc) as tc, ExitStack() as ctx:
        const = ctx.enter_context(tc.tile_pool(name="const", bufs=1))
        A_sb = const.tile([R, IF], bf16, name="A_sb")
        nc.sync.dma_start(A_sb[:], A[:])
        BT_sb = const.tile([R, OPC], bf16, name="BT_sb")
        nc.sync.dma_start(BT_sb[:], BT[:])
        id_sb = const.tile([P, NID * P], bf16, name="id_sb")
        nc.sync.dma_start(id_sb[:, 0:P], ident[:])
        # delta_j/2-scaled identities for the Sign levels, built on-chip
        for bi, j in enumerate(act_lv):
            nc.vector.tensor_scalar(
                id_sb[:, (1 + bi) * P : (2 + bi) * P],
                id_sb[:, 0:P],
                float(deltas[j]) / 2.0,
                None,
                op0=op.mult,
            )

        wrk = ctx.enter_context(tc.tile_pool(name="wrk", bufs=3))
        ub = ctx.enter_context(tc.tile_pool(name="ub", bufs=3))
        mk = ctx.enter_context(tc.tile_pool(name="mk", bufs=2 * NLVL + 2))
        qs = ctx.enter_context(tc.tile_pool(name="qs", bufs=2))
        qwp = ctx.enter_context(tc.tile_pool(name="qwp", bufs=KT))
        bab = ctx.enter_context(tc.tile_pool(name="bab", bufs=3))
        xp = ctx.enter_context(tc.tile_pool(name="xp", bufs=8))
        obp = ctx.enter_context(tc.tile_pool(name="obp", bufs=8))
        cps = ctx.enter_context(tc.tile_pool(name="cps", bufs=CH0, space="PSUM"))
        dqstack = ExitStack()
        dps = dqstack.enter_context(tc.tile_pool(name="dps", bufs=3, space="PSUM"))
        bps = dqstack.enter_context(tc.tile_pool(name="bps", bufs=1, space="PSUM"))

        # ---- DMA helpers
        stage = {}

        def emit_wstage(kt):
            s_sb = wrk.tile([P, 3 * OPC], f32, tag="wrm", name=f"wrm{kt}")
            nc.sync.dma_start(s_sb[:], wrm[kt * P : (kt + 1) * P, :])
            stage[kt] = s_sb

        x_store = {}

        def emit_x(key, kt, tt0_, width):
            if key not in x_store:
                xt = xp.tile([P, 2 * t_tile], bf16, tag="x", name=f"x_{key}")
                nc.sync.dma_start(
                    xt[:, 0 : width * t_tile],
                    xT[kt * P : (kt + 1) * P, tt0_ * t_tile : (tt0_ + width) * t_tile],
                )
                x_store[key] = (xt, tt0_, width)

        emit_wstage(0)
        emit_wstage(1)
        emit_x(("p0", 0), 0, 0, 1)

        ps0 = {
            c: cps.tile([P, t_tile], f32, tag="ps", name=f"ps0_{c[0]}_{c[1]}")
            for c in p0
        }
        qw_tiles = [None] * KT
        dq_st = [None] * KT
        ba_tiles = [None] * KT

        # ---- Phase 0: 2-deep pipeline — dequant ktile kt, finalize slab for
        # kt-1, pass-0 matmuls for kt-2. PE order per iter: lora(kt),
        # p0mm(kt-2), dve-level idmuls(kt), act-level idmuls(kt) — timed so PE
        # never waits on DVE mask / ACT sign production.
        for it in range(KT + 2):
            kt = it
            lvl = None
            if kt < KT:
                if kt + 2 < KT:
                    emit_wstage(kt + 2)
                if kt + 1 < KT:
                    emit_x(("p0", kt + 1), kt + 1, 0, 1)
                s_sb = stage.pop(kt)
                ksl = slice(kt * P, (kt + 1) * P)
                # LoRA tile: (lora_B @ lora_A).T[ksl, :] = A[:, ksl].T @ BT
                ba_ps = bps.tile([P, OPC], f32, tag="ba", name=f"baps{kt}")
                nc.tensor.matmul(ba_ps[:], A_sb[:, ksl], BT_sb[:], start=True, stop=True)
                # u = w * (1/max), rounded to bf16
                u_sb = ub.tile([P, OPC], bf16, tag="u", name=f"u{kt}")
                nc.vector.tensor_tensor(
                    u_sb[:], s_sb[:, 0:OPC], s_sb[:, OPC : 2 * OPC], op=op.mult
                )
                # staircase masks: DVE levels emit delta_j*(u > m_j) in bf16 (4x)
                lvl = []
                for j in dve_lv:
                    m = mk.tile([P, OPC], bf16, tag="mk", name=f"m{kt}_{j}")
                    nc.vector.tensor_scalar(
                        m[:], u_sb[:], float(mids[j]), float(deltas[j]),
                        op0=op.is_gt, op1=op.mult,
                    )
                    lvl.append((m, 0))
                # ACT levels: sign(u - m_j); delta_j/2 applied by scaled identity
                for bi, j in enumerate(act_lv):
                    s = mk.tile([P, OPC], bf16, tag="mk", name=f"s{kt}_{j}")
                    nc.scalar.activation(
                        s[:], u_sb[:], mybir.ActivationFunctionType.Sign,
                        bias=-float(mids[j]),
                    )
                    lvl.append((s, 1 + bi))
                # LoRA eviction on ACT after the signs (PSUM -> bf16 SBUF)
                ba_sb = bab.tile([P, OPC], bf16, tag="ba", name=f"ba{kt}")
                nc.scalar.copy(ba_sb[:], ba_ps[:])
                ba_tiles[kt] = ba_sb
            # pass-0 matmuls for kt-2 (PE-early: fills the mask-latency window)
            if it >= 2:
                pk2 = it - 2
                qwt2 = qw_tiles[pk2]
                xt, _, _ = x_store[("p0", pk2)]
                for tt, o in p0:
                    nc.tensor.matmul(
                        ps0[(tt, o)][:],
                        qwt2[:, o * P : (o + 1) * P],
                        xt[:, 0:t_tile],
                        start=(pk2 == 0),
                        stop=(pk2 == KT - 1),
                    )
                del x_store[("p0", pk2)]
            if kt < KT:
                # PE sums all level tiles into the dequant PSUM bank
                dq = dps.tile([P, OPC], f32, tag="dq", name=f"dq{kt}")
                for i, (m, blk) in enumerate(lvl):
                    nc.tensor.matmul(
                        dq[:], id_sb[:, blk * P : (blk + 1) * P], m[:],
                        start=(i == 0), stop=(i == len(lvl) - 1),
                    )
                dq_st[kt] = (dq, s_sb)
            if 1 <= it <= KT:
                pk = it - 1
                dq, s_sb_p = dq_st[pk]
                dq_st[pk] = None
                # qsc = (sum + C0) * max, then merge LoRA -> resident bf16 slab
                qsc = qs.tile([P, OPC], bf16, tag="qsc", name=f"qsc{pk}")
                nc.vector.scalar_tensor_tensor(
                    qsc[:], dq[:], C0, s_sb_p[:, 2 * OPC : 3 * OPC],
                    op0=op.add, op1=op.mult,
                )
                qwt = qwp.tile([P, OPC], bf16, tag="qwt", name=f"qw{pk}")
                nc.vector.tensor_tensor(qwt[:], qsc[:], ba_tiles[pk][:], op=op.add)
                ba_tiles[pk] = None
                qw_tiles[pk] = qwt

        # pass-0 evictions (DVE) + out DMA
        for tt, o in p0:
            o_sb = obp.tile([P, t_tile], f32, tag="o", name=f"ob0_{tt}_{o}")
            nc.vector.tensor_copy(o_sb[:], ps0[(tt, o)][:])
            nc.sync.dma_start(
                outT[o * P : (o + 1) * P, tt * t_tile : (tt + 1) * t_tile], o_sb[:]
            )

        # release dequant/lora PSUM banks, open 4 more chain banks
        dqstack.close()
        cps2 = ctx.enter_context(tc.tile_pool(name="cps2", bufs=CH - CH0, space="PSUM"))

        # ---- Remaining passes: CH chunks (2 token-tiles) each, slab resident
        steps = [(gi, kt) for gi in range(len(groups)) for kt in range(KT)]
        g_tt0 = [min(tt for tt, _ in g) for g in groups]
        g_w = [len({tt for tt, _ in g}) for g in groups]

        def prefetch(si):
            if si < len(steps):
                gi2, kt2 = steps[si]
                emit_x((gi2, kt2), kt2, g_tt0[gi2], g_w[gi2])

        prefetch(0)
        prefetch(1)
        prefetch(2)
        cur_ps = {}
        for si, (gi, kt) in enumerate(steps):
            if kt == 0:
                cur_ps = {}
                for ci, c in enumerate(groups[gi]):
                    pool = cps if ci < CH0 else cps2
                    cur_ps[c] = pool.tile(
                        [P, t_tile], f32, tag="ps", name=f"ps{gi}_{c[0]}_{c[1]}"
                    )
            prefetch(si + 3)
            xt, tt0_, _ = x_store[(gi, kt)]
            for tt, o in groups[gi]:
                co = (tt - tt0_) * t_tile
                nc.tensor.matmul(
                    cur_ps[(tt, o)][:],
                    qw_tiles[kt][:, o * P : (o + 1) * P],
                    xt[:, co : co + t_tile],
                    start=(kt == 0),
                    stop=(kt == KT - 1),
                )
            del x_store[(gi, kt)]
            if kt == KT - 1:
                for tt, o in groups[gi]:
                    o_sb = obp.tile([P, t_tile], f32, tag="o", name=f"obg{gi}_{tt}_{o}")
                    nc.vector.tensor_copy(o_sb[:], cur_ps[(tt, o)][:])
                    nc.sync.dma_start(
                        outT[o * P : (o + 1) * P, tt * t_tile : (tt + 1) * t_tile],
                        o_sb[:],
                    )

    nc.compile()
    return nc


def _lut_consts(lookup_table):
    lut = np.asarray(lookup_table, np.float64)
    mids = ((lut[:-1] + lut[1:]) / 2).astype(np.float32)
    deltas = (lut[1:] - lut[:-1]).astype(np.float32)
    c0 = np.float32(lut[0])
    return mids, deltas, c0


def prep_inputs(x, weight, lora_A, lora_B, max_val, mode, n_cores=N_CORES):
    """Host-side sharding/layout prep. Returns in_maps (one dict per core)."""
    f32 = np.float32
    bf16 = _np_dt(mybir.dt.bfloat16)
    T, IF = x.shape
    OF = weight.shape[0]
    OPC = OF // n_cores

    xT = np.ascontiguousarray(np.asarray(x, f32).T).astype(bf16)
    A = np.ascontiguousarray(np.asarray(lora_A, f32)).astype(bf16)
    maxR = np.asarray(max_val, f32).reshape(OF, IF // BLOCK)  # [o, block]
    w = np.asarray(weight, f32)
    B = np.asarray(lora_B, f32)

    in_maps = []
    for c in range(n_cores):
        osl = slice(c * OPC, (c + 1) * OPC)
        wT_c = np.ascontiguousarray(w[osl].T)  # [IF, OPC]
        mx_c = np.repeat(maxR[osl].T, BLOCK, axis=0)  # [IF, OPC]
        rb_c = (f32(1.0) / mx_c).astype(f32)
        wrm = np.concatenate([wT_c, rb_c, mx_c], axis=1)  # [IF, 3*OPC]
        in_maps.append(
            {
                "ident": np.eye(P, dtype=bf16),
                "xT": xT,
                "wrm": np.ascontiguousarray(wrm),
                "A": A,
                "BT": np.ascontiguousarray(B[osl].T).astype(bf16),  # [R, OPC]
            }
        )
    return in_maps


def _get_program(mids, deltas, c0, mode):
    key = (
        mode,
        tuple(np.asarray(mids).tolist()),
        tuple(np.asarray(deltas).tolist()),
        float(c0),
    )
    if key not in _CACHE:
        _CACHE[key] = build_program(
            T_FULL, IN_F, OUT_F // N_CORES, RANK, N_CORES, mids, deltas, c0, mode
        )
    return _CACHE[key]


def kernel(x, weight, lora_A, lora_B, max_val, lookup_table):
    mids, deltas, c0 = _lut_consts(lookup_table)
    nc = _get_program(mids, deltas, c0, MODE)
    in_maps = prep_inputs(x, weight, lora_A, lora_B, max_val, MODE)
    res = run_bass_kernel_spmd(nc, in_maps, core_ids=list(range(N_CORES))).results
    outT = np.concatenate([res[c]["outT"] for c in range(N_CORES)], axis=0)  # [OF, T]
    return np.ascontiguousarray(outT.T).astype(np.float32)
